# revision 22
# baseline (speedup 1.0000x reference)
"""Trainium2 Bass kernel for nn_AttentionLayer (B=4, N=4096, D=1024).

Reference computation:
  nx = layernorm(x)
  h  = nx @ expand                       # [B,N,4352]
  q  = h[:, :128] ; k = h[:, 128:256]
  linear = h[:, 256:2304]; pre_gelu = h[:, 2304:4352]
  gated  = linear * gelu(pre_gelu)       # exact erf gelu
  local  = gated[:, :1024]; v = gated[:, 1024:2048]
  mask[i,j] = j<=i ? sigmoid((j-i)+pbm) : -inf
  attn = softmax(q k^T / sqrt(128) + mask) @ v
  out  = x + concat([local, attn]) @ project

Sharding (8 cores, SPMD): batch b -> core pair (2b, 2b+1).  Per pair,
512-row query blocks interleave for causal load balance: even core owns
blocks {0,3,4,7}, odd owns {1,2,5,6}.  Each core computes LN + expand for
its OWN 2048 rows only; k/v of the other half arrive via pairwise
AllGathers (one k + one v collective per 512-row chunk, issued as each
chunk lands in HBM so the wire time pipelines under the remaining
expand).  The kv slot order is the fixed pair order
[even-core blocks | odd-core blocks], the same on both cores, so the
SPMD attention schedule is uniform: q-slot i attends a fixed slot set
(2/4/6/8 slots).  Causality + position bias use a host-precomputed
multiplicative mask expM = causal ? exp(sigmoid(j-i+pbm)) : 0, but the
mask is only loaded/applied on the 11 (q-slot, kv-slot) pairs where it
differs from 1.0 on either core of the pair (diagonal blocks, band
precursors, and fully-future blocks); elsewhere expM == 1 to bf16
precision because sigmoid(j-i+pbm) underflows ~16 columns past the
diagonal.  P = exp(qk)*expM is normalized by its row sum (no max
subtraction: logits are O(1) after layernorm + xavier weights).

Precision: fp8(e4m3) DoubleRow matmuls (2x PE throughput) for the
expand, attention*V and project matmuls, with per-tensor power-of-2
scales folded into psum-evacuation constants; qk^T stays bf16.  psum
accumulation is f32 throughout.  Measured end-to-end rel err ~1.2e-2
(tolerance 2e-2).

Schedule notes: all x^T tiles are preloaded up front (no per-chunk DMA
dependency chains); DMAs are issued as single multi-dim descriptors
(host pre-transposes arrays into [128, chunk, cols] layouts) to keep
the sync-engine descriptor-issue time off the critical path; psum
evacuations that feed a multiply are fused into one
scalar_tensor_tensor; the attention denominator matmul is emitted after
the first AV psum group so the PE never waits on the DVE add chain.
"""

import math

import numpy as np
import ml_dtypes

import concourse.bass as bass
import concourse.mybir as mybir
from concourse import bacc
import concourse.tile as tile
from concourse.bass_utils import run_bass_kernel_spmd

BF16 = mybir.dt.bfloat16
F32 = mybir.dt.float32
FP8 = mybir.dt.float8e4
AF = mybir.ActivationFunctionType
PM = mybir.MatmulPerfMode
MUL = mybir.AluOpType.mult
ADD = mybir.AluOpType.add

B, N, D = 4, 4096, 1024
QK = 128
E = 2048
R = N              # kv rows per core
RO = 2048          # own query rows per core
DCH = D // 128     # 8 contraction chunks
NT = 512           # matmul free-dim tile
W2 = 2176          # 128 (q or k) + 1024 (linear) + 1024 (gelu) cols

# fp8 scales (powers of 2; relative precision is scale-free, these just
# center the dynamic range away from subnormals/overflow)
SX = 16.0          # nx (post-LN activations)
SW = 256.0         # wk / wlin / wgel columns
SWQ = 4096.0       # wq columns (also absorbs the 1/sqrt(qk) prescale)
SV = 8.0           # v / local / attn (the project stationary operands)
SWP = 256.0        # wproj
LN4 = math.log(4.0)  # exp bias => P scaled by 4
USK = 1.0 / (SX * SW)    # 2^-12: k / linear / gelu psum evacuation
USQ = 1.0 / (SX * SWQ)   # 2^-16: q psum evacuation
USO = 1.0 / (SV * SWP)   # 2^-11: project psum evacuation
# NOTE: attnT = av_psum/den_psum exactly: av = sum (4P)(8v) = 32*sum(Pv),
# den = 4*sum(P), so av/den = 8*attn = SV*attn as required.

# attention schedule: q-slot qi attends kv slots 0..qi and 4..4+qi
SCHED = {0: [0, 4], 1: [0, 1, 4, 5], 2: [0, 1, 2, 4, 5, 6], 3: [0, 1, 2, 3, 4, 5, 6, 7]}
# (qi, slot) pairs whose mask differs from all-ones on either core of the
# pair: diagonal blocks, band precursors (j >= i-16 tail), future blocks
MASKED = {0: (0, 4), 1: (1, 4, 5), 2: (1, 2, 6), 3: (3, 6, 7)}
MIDX = {}
for _qi in range(4):
    for _s in MASKED[_qi]:
        MIDX[(_qi, _s)] = len(MIDX)
NMSK = len(MIDX)   # 11

LAST_RESULTS = None  # set by kernel(); test harness reads exec_time_ns


def _build_nc():
    nc = bacc.Bacc(None)

    # host pre-transposed layouts: leading dim is the SBUF partition
    xt = nc.declare_dram_parameter("xt", [128, DCH, RO], BF16, isOutput=False)
    xo = nc.declare_dram_parameter("xo", [RO, D], BF16, isOutput=False)
    wkv = nc.declare_dram_parameter("wkv", [128, DCH, W2], FP8, isOutput=False)
    wql = nc.declare_dram_parameter("wql", [128, DCH, W2], FP8, isOutput=False)
    wproj = nc.declare_dram_parameter("wproj", [128, 16, D], FP8, isOutput=False)
    msk = nc.declare_dram_parameter("msk", [128, NMSK, 4, NT], BF16, isOutput=False)
    out = nc.declare_dram_parameter("out", [RO, D], F32, isOutput=True)

    with tile.TileContext(nc) as tc:
        with tc.tile_pool(name="const", bufs=1) as cpool:
            ones128 = cpool.tile([128, 1], BF16)
            nc.vector.memset(ones128[:], 1.0)
            ones1 = cpool.tile([1, 128], BF16)
            nc.vector.memset(ones1[:], 1.0)
            ln4b = cpool.tile([128, 1], F32)
            nc.vector.memset(ln4b[:], LN4)
            epsb = cpool.tile([128, 1], F32)
            nc.vector.memset(epsb[:], 1e-5 / (SX * SX))

            with tc.tile_pool(name="dram", bufs=1, space="DRAM") as dpool:
                kv_own = [dpool.tile([128, 4, 1024], FP8, name=f"kv_own_{r}")
                          for r in range(4)]
                kv_all = [dpool.tile([2 * 128, 4, 1024], FP8, name=f"kv_all_{r}")
                          for r in range(4)]
                k_own = [dpool.tile([128, NT], BF16, name=f"k_own_{r}")
                         for r in range(4)]
                k_all = [dpool.tile([2 * 128, NT], BF16, name=f"k_all_{r}")
                         for r in range(4)]

                with tc.tile_pool(name="persist", bufs=1) as ppool, \
                     tc.tile_pool(name="attnT_p", bufs=1) as apool, \
                     tc.tile_pool(name="q0_p", bufs=1) as q0pool:
                    kT_sb = ppool.tile([128, R], BF16)         # k^T, hT layout
                    qT_sb = ppool.tile([128, RO], BF16)        # q^T
                    localT_sb = ppool.tile([128, 8, RO], FP8)  # [lc][128, 2048] *SV

                    # attention-phase SBUF pools (psbp/astream/awork) are the
                    # outer-scope pools above so q-slot 0's psb production can
                    # be woven into loop2 (its qk matmuls borrow the expand
                    # psum pool)
                    def make_producer(qi, pools):
                        """psb production for q-slot qi: per tile one qk
                        matmul + exp (+mask mul) + split den add.  Returned
                        as (state, generator) so the caller can weave single
                        tiles of production between other matmul groups: the
                        PE then always has work while the scalar-engine exp
                        chain paces production."""
                        kr_slots = SCHED[qi]
                        qcol = qi * NT
                        psb_pool, pt_pool, pt_tag, pt_bufs, pe_pool, mt_pool = pools
                        st = {
                            "psb": psb_pool.tile([128, len(kr_slots) * 4, NT], FP8,
                                                 tag=f"psb{qi}", bufs=1,
                                                 name=f"psb_{qi}"),
                            "den_a": psb_pool.tile([128, NT], BF16, tag=f"dena{qi}",
                                                   bufs=1, name=f"den_a_{qi}"),
                            "den_b": psb_pool.tile([128, NT], BF16, tag=f"denb{qi}",
                                                   bufs=1, name=f"den_b_{qi}"),
                        }

                        def gen():
                            psb = st["psb"]
                            for i, krs in enumerate(kr_slots):
                                mt4 = None
                                if (qi, krs) in MIDX:
                                    mt4 = mt_pool.tile([128, 4, NT], BF16,
                                                       tag="mt", bufs=4)
                                    nc.sync.dma_start(
                                        mt4[:], msk[:, MIDX[(qi, krs)], :, :])
                                for j in range(4):
                                    ti = i * 4 + j
                                    kr0 = krs * NT + j * 128
                                    pt_ps = pt_pool.tile([128, NT], F32,
                                                         tag=pt_tag, bufs=pt_bufs)
                                    nc.tensor.matmul(pt_ps[:],
                                                     kT_sb[:, kr0:kr0 + 128],
                                                     qT_sb[:, qcol:qcol + NT],
                                                     start=True, stop=True)
                                    if mt4 is not None:
                                        pe = pe_pool.tile([128, NT], BF16,
                                                          tag="pe", bufs=4)
                                        nc.scalar.activation(pe[:], pt_ps[:],
                                                             AF.Exp, bias=ln4b[:])
                                        nc.vector.tensor_mul(psb[:, ti, :], pe[:],
                                                             mt4[:, j, :])
                                    else:
                                        nc.scalar.activation(psb[:, ti, :],
                                                             pt_ps[:], AF.Exp,
                                                             bias=ln4b[:])
                                    acc = st["den_a"] if ti % 2 == 0 else st["den_b"]
                                    if ti < 2:
                                        nc.vector.tensor_copy(acc[:], psb[:, ti, :])
                                    else:
                                        nc.vector.tensor_add(acc[:], acc[:],
                                                             psb[:, ti, :])
                                    yield
                        return st, gen()

                    def weave(nxt, k):
                        if nxt is None:
                            return
                        for _ in range(k):
                            if next(nxt, "DONE") == "DONE":
                                break

                    # ---------------- Phase 1+2: expand ----------------
                    with tc.tile_pool(name="xt_p", bufs=1) as xtp, \
                         tc.tile_pool(name="wkv_p", bufs=1) as wkvp, \
                         tc.tile_pool(name="wql_p", bufs=1) as wqlp, \
                         tc.tile_pool(name="ex_stream", bufs=4) as estream, \
                         tc.tile_pool(name="ex_work", bufs=3) as ework, \
                         tc.tile_pool(name="st_work", bufs=2) as swork, \
                         tc.tile_pool(name="ex_psum", bufs=5, space="PSUM") as epsum, \
                         tc.tile_pool(name="st_psum", bufs=2, space="PSUM") as spsum:
                        # preload ALL x^T tiles + weights up front; chunk 0's x
                        # first (feeds the first stats chain), then wkv (first
                        # matmuls), remaining x, then wql (needed only in loop2)
                        xt_all = [xtp.tile([128, DCH, NT], BF16, name=f"xt_{r}")
                                  for r in range(4)]
                        wkv_sb = wkvp.tile([128, DCH, W2], FP8)
                        wql_sb = wqlp.tile([128, DCH, W2], FP8)
                        for h in range(2):
                            nc.sync.dma_start(xt_all[0][:, 4 * h:4 * h + 4, :],
                                              xt[:, 4 * h:4 * h + 4, 0:NT])
                        for dq in range(4):
                            nc.sync.dma_start(wkv_sb[:, 2 * dq:2 * dq + 2, :],
                                              wkv[:, 2 * dq:2 * dq + 2, :])
                        for r in range(1, 4):
                            for h in range(2):
                                nc.sync.dma_start(
                                    xt_all[r][:, 4 * h:4 * h + 4, :],
                                    xt[:, 4 * h:4 * h + 4, r * NT:(r + 1) * NT])
                        for dq in range(4):
                            nc.sync.dma_start(wql_sb[:, 2 * dq:2 * dq + 2, :],
                                              wql[:, 2 * dq:2 * dq + 2, :])

                        def stats_chain(rch):
                            """Raw LN sums for chunk rch from the preloaded x^T
                            tiles: partition-sum matmuls + scalar-engine psum
                            evacuation.  Emitted one iteration ahead so the DVE
                            adds hide under the previous chunk's expand."""
                            xts = xt_all[rch]
                            mu_ps = spsum.tile([1, NT], F32, tag="stat", name=f"mu_ps_{rch}")
                            sq_ps = spsum.tile([1, NT], F32, tag="stat", name=f"sq_ps_{rch}")
                            # accumulate the 8 d-chunks on DVE (bf16 2x mode), then a
                            # single partition-sum matmul per stat instead of 8 each
                            acc_mu = estream.tile([128, NT], BF16, tag="acc_mu", bufs=2,
                                                  name=f"accmu_{rch}")
                            acc_sq = estream.tile([128, NT], BF16, tag="acc_sq", bufs=2,
                                                  name=f"accsq_{rch}")
                            sq_prev = estream.tile([128, NT], BF16, tag="sq_s", bufs=2,
                                                    name=f"sq_{rch}_0")
                            nc.gpsimd.tensor_mul(sq_prev[:], xts[:, 0, :], xts[:, 0, :])
                            nc.vector.tensor_add(acc_mu[:], xts[:, 0, :], xts[:, 1, :])
                            for dch in range(1, DCH):
                                sqt = estream.tile([128, NT], BF16, tag="sq_s", bufs=2,
                                                   name=f"sq_{rch}_{dch}")
                                nc.gpsimd.tensor_mul(sqt[:], xts[:, dch, :], xts[:, dch, :])
                                if dch == 1:
                                    nc.vector.tensor_add(acc_sq[:], sq_prev[:], sqt[:])
                                else:
                                    nc.vector.tensor_add(acc_sq[:], acc_sq[:], sqt[:])
                                if dch >= 2:
                                    nc.vector.tensor_add(acc_mu[:], acc_mu[:], xts[:, dch, :])
                            nc.tensor.matmul(mu_ps[:], ones128[:], acc_mu[:],
                                             start=True, stop=True)
                            nc.tensor.matmul(sq_ps[:], ones128[:], acc_sq[:],
                                             start=True, stop=True)
                            mu16 = swork.tile([1, NT], BF16, tag="st_m16", bufs=1, name=f"m16_{rch}")
                            e16 = swork.tile([1, NT], BF16, tag="st_e16", bufs=1, name=f"e16_{rch}")
                            nc.vector.tensor_scalar_mul(mu16[:], mu_ps[:], 1.0 / D)
                            nc.vector.tensor_scalar_mul(e16[:], sq_ps[:], 1.0 / D)
                            return mu16, e16

                        def bcast_chain(rch, mu16, e16):
                            # broadcast the raw mean / second moment to 128
                            # partitions FIRST, then do the LN scale/shift math
                            # at [128,NT] width (DVE [1,N] ops cost the same
                            # cycles as [128,N] ones, so broadcasting early is
                            # free and keeps the chain short)
                            bps = spsum.tile([128, NT], F32, tag="bcast", bufs=1, name=f"bps_{rch}")
                            nc.tensor.matmul(bps[:], ones1[:], mu16[:], start=True, stop=True)
                            mub = swork.tile([128, NT], BF16, tag="mub", bufs=2,
                                             name=f"mub_{rch}")
                            nc.vector.tensor_copy(mub[:], bps[:])
                            bps2 = spsum.tile([128, NT], F32, tag="bcast", bufs=1, name=f"bps2_{rch}")
                            nc.tensor.matmul(bps2[:], ones1[:], e16[:], start=True, stop=True)
                            e2b = swork.tile([128, NT], BF16, tag="e2b", bufs=2,
                                             name=f"e2b_{rch}")
                            nc.vector.tensor_copy(e2b[:], bps2[:])
                            var = swork.tile([128, NT], BF16, tag="var", bufs=1,
                                             name=f"var_{rch}")
                            nc.vector.tensor_mul(var[:], mub[:], mub[:])
                            nc.vector.tensor_sub(var[:], e2b[:], var[:])
                            # rstd_bt = SX/sqrt(var+eps) = 1/sqrt(var/SX^2 + eps/SX^2)
                            s_f = swork.tile([128, NT], F32, tag="s_f", bufs=1,
                                             name=f"s_f_{rch}")
                            nc.scalar.activation(s_f[:], var[:], AF.Sqrt,
                                                 scale=1.0 / (SX * SX), bias=epsb[:])
                            r_f = swork.tile([128, NT], F32, tag="r_f", bufs=1,
                                             name=f"r_f_{rch}")
                            nc.vector.reciprocal_approx_fast(r_f[:], s_f[:])
                            rstd_bt = swork.tile([128, NT], BF16, tag="rbt", bufs=2,
                                                 name=f"rbt_{rch}")
                            nc.vector.tensor_copy(rstd_bt[:], r_f[:])
                            sneg_bt = swork.tile([128, NT], BF16, tag="sbt", bufs=2,
                                                 name=f"sbt_{rch}")
                            nc.vector.scalar_tensor_tensor(
                                sneg_bt[:], mub[:], -1.0, rstd_bt[:], op0=MUL, op1=MUL)
                            return rstd_bt, sneg_bt

                        def center_chain(rch, rstd_bt, sneg_bt):
                            # xpp = SX*(x*rstd - mu*rstd) in fp8, [128, DCH, NT];
                            # emitted mid-way through the PREVIOUS chunk's expand
                            xts = xt_all[rch]
                            xpp = estream.tile([128, DCH, NT], FP8, tag="xpp", bufs=4,
                                               name=f"xpp_{rch}")
                            for dch in range(DCH):
                                xc = ework.tile([128, NT], BF16, tag="cen", bufs=3,
                                                name=f"cen_{rch}_{dch}")
                                nc.gpsimd.tensor_mul(xc[:], xts[:, dch, :], rstd_bt[:])
                                nc.vector.tensor_add(xpp[:, dch, :], xc[:], sneg_bt[:])
                            return xpp

                        def v_group(rch, xpp, ms):
                            for m in ms:
                                vgel = ework.tile([128, E // 2], BF16, tag="vgel")
                                vv = ework.tile([128, E // 2], FP8, tag="vv")
                                # gelu columns first so the fused lin*gelu stt
                                # has its second operand ready
                                for vc in (2, 3, 0, 1):
                                    vps = epsum.tile([128, NT], F32, tag="mm")
                                    if vc < 2:
                                        woff = 128 + vc * NT
                                    else:
                                        woff = 1152 + (vc - 2) * NT
                                    for dp in range(DCH // 2):
                                        nc.tensor.matmul(
                                            vps[:],
                                            xpp[:, 2 * dp:2 * dp + 2, m * 128:(m + 1) * 128],
                                            wkv_sb[:, 2 * dp:2 * dp + 2, woff:woff + NT],
                                            start=(dp == 0), stop=(dp == DCH // 2 - 1),
                                            perf_mode=PM.DoubleRow)
                                    if vc >= 2:
                                        nc.scalar.activation(vgel[:, (vc - 2) * NT:(vc - 1) * NT],
                                                             vps[:], AF.Gelu, scale=USK)
                                    else:
                                        # vv = (lin_psum * USK*SV) * gelu, fused
                                        nc.vector.scalar_tensor_tensor(
                                            vv[:, vc * NT:(vc + 1) * NT], vps[:],
                                            USK * SV, vgel[:, vc * NT:(vc + 1) * NT],
                                            op0=MUL, op1=MUL)
                                nc.sync.dma_start(kv_own[rch][:, m, :], vv[:])

                        rch_order = [0, 1, 2, 3]

                        st0 = stats_chain(rch_order[0])
                        bt0 = bcast_chain(rch_order[0], st0[0], st0[1])
                        xpp_stash = center_chain(rch_order[0], bt0[0], bt0[1])
                        xpps = {}
                        NOWN = 4
                        for oi, rch in enumerate(rch_order):
                            xpp = xpp_stash
                            xpps[rch] = xpp
                            if oi + 1 < NOWN:
                                nxt = stats_chain(rch_order[oi + 1])
                            # k^T own (hT layout) -> DRAM bounce for the AllGather
                            kps = epsum.tile([128, NT], F32, tag="mm")
                            for dp in range(DCH // 2):
                                nc.tensor.matmul(kps[:],
                                                 wkv_sb[:, 2 * dp:2 * dp + 2, 0:128],
                                                 xpp[:, 2 * dp:2 * dp + 2, :],
                                                 start=(dp == 0), stop=(dp == DCH // 2 - 1),
                                                 perf_mode=PM.DoubleRow)
                            kout = ework.tile([128, NT], BF16, tag="kout", bufs=2,
                                              name=f"kout_{rch}")
                            nc.vector.tensor_scalar_mul(kout[:], kps[:], USK)
                            nc.sync.dma_start(k_own[rch][:], kout[:])
                            v_group(rch, xpp, (0, 1))
                            if oi + 1 < NOWN:
                                nbt = bcast_chain(rch_order[oi + 1], nxt[0], nxt[1])
                                xpp_stash = center_chain(rch_order[oi + 1],
                                                         nbt[0], nbt[1])
                            v_group(rch, xpp, (2, 3))
                            # AllGathers for this chunk's k and v: issued as soon as
                            # the chunk is in HBM so the wire time pipelines under
                            # the remaining expand
                            nc.gpsimd.collective_compute(
                                "AllGather",
                                mybir.AluOpType.bypass,
                                replica_groups=[[0, 1], [2, 3], [4, 5], [6, 7]],
                                ins=[k_own[rch].opt()],
                                outs=[k_all[rch].opt()],
                            )
                            nc.gpsimd.collective_compute(
                                "AllGather",
                                mybir.AluOpType.bypass,
                                replica_groups=[[0, 1], [2, 3], [4, 5], [6, 7]],
                                ins=[kv_own[rch].opt()],
                                outs=[kv_all[rch].opt()],
                            )
                            # kv slot order is [rank0 chunks | rank1 chunks]
                            nc.sync.dma_start(kT_sb[:, rch * NT:(rch + 1) * NT],
                                              k_all[rch][0:128, :])
                            nc.sync.dma_start(kT_sb[:, (4 + rch) * NT:(5 + rch) * NT],
                                              k_all[rch][128:256, :])
                        # loop2: q + local expand (covers the AllGather latency)
                        # q-slot 0's psb production weaves into loop2 chunks 1-3
                        # (qT slot 0 is written by chunk 0's q evacuation);
                        # its qk matmuls borrow the expand psum pool
                        st0, pgen0 = make_producer(
                            0, (q0pool, epsum, "mm", 5, ework, estream))
                        for rch in rch_order:
                            rs = rch * NT
                            xpp = xpps[rch]
                            qps = epsum.tile([128, NT], F32, tag="mm")
                            for dp in range(DCH // 2):
                                nc.tensor.matmul(qps[:],
                                                 wql_sb[:, 2 * dp:2 * dp + 2, 0:128],
                                                 xpp[:, 2 * dp:2 * dp + 2, :],
                                                 start=(dp == 0), stop=(dp == DCH // 2 - 1),
                                                 perf_mode=PM.DoubleRow)
                            nc.vector.tensor_scalar_mul(qT_sb[:, rs:rs + NT],
                                                        qps[:], USQ)
                            if rch == 3:
                                weave(pgen0, 99)
                            for lc in range(8):
                                lps = epsum.tile([128, NT], F32, tag="mm")
                                gps = epsum.tile([128, NT], F32, tag="mm")
                                for dp in range(DCH // 2):
                                    nc.tensor.matmul(
                                        gps[:],
                                        wql_sb[:, 2 * dp:2 * dp + 2,
                                               1152 + lc * 128:1280 + lc * 128],
                                        xpp[:, 2 * dp:2 * dp + 2, :],
                                        start=(dp == 0), stop=(dp == DCH // 2 - 1),
                                        perf_mode=PM.DoubleRow)
                                for dp in range(DCH // 2):
                                    nc.tensor.matmul(
                                        lps[:],
                                        wql_sb[:, 2 * dp:2 * dp + 2,
                                               128 + lc * 128:256 + lc * 128],
                                        xpp[:, 2 * dp:2 * dp + 2, :],
                                        start=(dp == 0), stop=(dp == DCH // 2 - 1),
                                        perf_mode=PM.DoubleRow)
                                lgel = ework.tile([128, NT], BF16, tag="lgel")
                                nc.scalar.activation(lgel[:], gps[:], AF.Gelu, scale=USK)
                                # localT = (lin_psum * USK*SV) * gelu, fused
                                nc.vector.scalar_tensor_tensor(
                                    localT_sb[:, lc, rs:rs + NT], lps[:], USK * SV,
                                    lgel[:], op0=MUL, op1=MUL)
                        # finish any q-slot 0 production not woven into loop2
                        # (still inside the expand pools: its qk matmuls use
                        # the expand psum ring)
                        weave(pgen0, 99)

                    # ---------------- Phase 3: attention ----------------
                    # allocated only now so the expand phase has the SBUF;
                    # project weights prefetch during attention
                    attnT_sb = apool.tile([128, 8, RO], FP8)   # [vc][128, 2048] *SV
                    proj_sb = apool.tile([128, 16, D], FP8)
                    for pq in range(4):
                        nc.sync.dma_start(proj_sb[:, 4 * pq:4 * pq + 4, :],
                                          wproj[:, 4 * pq:4 * pq + 4, :])
                    with tc.tile_pool(name="psb_p", bufs=1) as psbp, \
                         tc.tile_pool(name="at_stream", bufs=6) as astream, \
                         tc.tile_pool(name="at_work", bufs=2) as awork, \
                         tc.tile_pool(name="at_psum", bufs=4, space="PSUM") as apsum, \
                         tc.tile_pool(name="av_psum", bufs=2, space="PSUM") as avpsum:

                        def consume(qi, st, nxt):
                            kr_slots = SCHED[qi]
                            qcol = qi * NT
                            nkr = len(kr_slots) * 4
                            psb = st["psb"]
                            nslots = len(kr_slots)
                            # weave qi+1's production into the AV groups:
                            # lightly during g0 (so the den->reciprocal chain
                            # isn't stuck behind woven DVE work), heavier in g1
                            kw0 = 2 if qi < 3 else 0
                            kw1 = 4 if qi < 3 else 0
                            rd_b = None
                            for g in range(2):
                                avs = [avpsum.tile([128, NT], F32, tag="av", bufs=4,
                                                   name=f"av{g}_{_i}")
                                       for _i in range(4)]
                                for i, krs in enumerate(kr_slots):
                                    gslot, vrank = krs % 4, krs // 4
                                    vt4 = astream.tile([128, 4, NT], FP8, tag="vt",
                                                       bufs=6)
                                    nc.sync.dma_start(
                                        vt4[:],
                                        kv_all[gslot][vrank * 128:(vrank + 1) * 128,
                                                      :, g * NT:(g + 1) * NT])
                                    for jp in range(2):
                                        ti0 = i * 4 + 2 * jp
                                        for v4 in range(4):
                                            nc.tensor.matmul(
                                                avs[v4][:],
                                                vt4[:, 2 * jp:2 * jp + 2,
                                                    v4 * 128:(v4 + 1) * 128],
                                                psb[:, ti0:ti0 + 2, :],
                                                start=(ti0 == 0), stop=(ti0 == nkr - 2),
                                                perf_mode=PM.DoubleRow)
                                    weave(nxt, kw0 if g == 0 else kw1)
                                if g == 0:
                                    den_ps = apsum.tile([1, NT], F32, tag="den",
                                                        bufs=1)
                                    nc.tensor.matmul(den_ps[:], ones128[:],
                                                     st["den_a"][:],
                                                     start=True, stop=False)
                                    nc.tensor.matmul(den_ps[:], ones128[:],
                                                     st["den_b"][:],
                                                     start=False, stop=True)
                                    # den evacuation + rd broadcast evacuation on
                                    # the scalar engine: keeps the reciprocal's
                                    # feed/drain off the busy DVE queue
                                    den = awork.tile([1, NT], F32, tag="den_sb")
                                    rec = awork.tile([1, NT], F32, tag="rec")
                                    rec16 = awork.tile([1, NT], BF16, tag="rec16")
                                    nc.vector.tensor_copy(den[:], den_ps[:])
                                    nc.vector.reciprocal_approx_fast(rec[:], den[:])
                                    nc.vector.tensor_copy(rec16[:], rec[:])
                                    weave(nxt, 3)
                                    rb_ps = apsum.tile([128, NT], F32, tag="pt",
                                                       bufs=3)
                                    nc.tensor.matmul(rb_ps[:], ones1[:], rec16[:],
                                                     start=True, stop=True)
                                    rd_b = awork.tile([128, NT], BF16, tag="rd_b")
                                    nc.vector.tensor_copy(rd_b[:], rb_ps[:])
                                for v4 in range(4):
                                    nc.vector.tensor_mul(
                                        attnT_sb[:, g * 4 + v4, qcol:qcol + NT],
                                        avs[v4][:], rd_b[:])
                            weave(nxt, 99)  # drain any remainder

                        st = st0
                        for qi in range(4):
                            nxt_st = nxt_gen = None
                            if qi < 3:
                                nxt_st, nxt_gen = make_producer(
                                    qi + 1, (psbp, apsum, "pt", 3, awork, astream))
                            consume(qi, st, nxt_gen)
                            st = nxt_st

                    # ---------------- Phase 4: project + residual ----------------
                    with tc.tile_pool(name="pr_stream", bufs=4) as prstream, \
                         tc.tile_pool(name="pr_psum", bufs=4, space="PSUM") as prpsum:
                        for rt in range(RO // 128):
                            xo_t = prstream.tile([128, D], BF16, tag="xo")
                            nc.sync.dma_start(xo_t[:], xo[rt * 128:(rt + 1) * 128, :])
                            ot = prstream.tile([128, D], F32, tag="ot")
                            for dc in range(2):
                                ops = prpsum.tile([128, NT], F32, tag="out")
                                for lp in range(4):
                                    nc.tensor.matmul(
                                        ops[:],
                                        localT_sb[:, 2 * lp:2 * lp + 2,
                                                  rt * 128:(rt + 1) * 128],
                                        proj_sb[:, 2 * lp:2 * lp + 2,
                                                dc * NT:(dc + 1) * NT],
                                        start=(lp == 0), stop=False,
                                        perf_mode=PM.DoubleRow)
                                for ap_ in range(4):
                                    nc.tensor.matmul(
                                        ops[:],
                                        attnT_sb[:, 2 * ap_:2 * ap_ + 2,
                                                 rt * 128:(rt + 1) * 128],
                                        proj_sb[:, 8 + 2 * ap_:10 + 2 * ap_,
                                                dc * NT:(dc + 1) * NT],
                                        start=False, stop=(ap_ == 3),
                                        perf_mode=PM.DoubleRow)
                                nc.vector.scalar_tensor_tensor(
                                    ot[:, dc * NT:(dc + 1) * NT], ops[:], USO,
                                    xo_t[:, dc * NT:(dc + 1) * NT],
                                    op0=MUL, op1=ADD)
                                nc.sync.dma_start(
                                    out[rt * 128:(rt + 1) * 128, dc * NT:(dc + 1) * NT],
                                    ot[:, dc * NT:(dc + 1) * NT])

    nc.compile()
    return nc


_ORDERS = {0: [0, 3, 4, 7, 1, 2, 5, 6], 1: [1, 2, 5, 6, 0, 3, 4, 7]}


def _sigmoid(x):
    return np.where(x >= 0, 1.0 / (1.0 + np.exp(-np.abs(x))),
                    np.exp(-np.abs(x)) / (1.0 + np.exp(-np.abs(x))))


def _chunk_part(a, nch):
    """[nch*128, C] -> [128, nch, C] with [p, i, c] = a[i*128+p, c]."""
    return np.ascontiguousarray(
        a.reshape(nch, 128, a.shape[1]).transpose(1, 0, 2))


def _prep_inputs(x, expand, project, pbm):
    """Build per-core input maps (host-side sharding)."""
    bf16 = ml_dtypes.bfloat16
    fp8 = ml_dtypes.float8_e4m3
    sc = 1.0 / math.sqrt(QK)
    wq = expand[:, :QK] * (sc * SWQ)
    wk = expand[:, QK:2 * QK] * SW
    lin = expand[:, 2 * QK:2 * QK + E] * SW
    gel = expand[:, 2 * QK + E:] * SW
    wkv = _chunk_part(
        np.concatenate([wk, lin[:, D:], gel[:, D:]], axis=1), DCH).astype(fp8)
    wql = _chunk_part(
        np.concatenate([wq, lin[:, :D], gel[:, :D]], axis=1), DCH).astype(fp8)
    wproj = _chunk_part(project * SWP, 16).astype(fp8)

    in_maps = []
    NBQ = 512
    for c in range(8):
        b, half = c // 2, c % 2
        order = _ORDERS[half]
        xb = x[b]
        xperm = np.concatenate([xb[blk * NBQ:(blk + 1) * NBQ] for blk in order[:4]], axis=0)
        xt = _chunk_part(np.ascontiguousarray(xperm.T), DCH).astype(bf16)  # [128, DCH, 2048]
        xo = np.ascontiguousarray(xperm).astype(bf16)
        # kv slots in FIXED pair order: [A blocks 0,3,4,7 | B blocks 1,2,5,6]
        kv_order = _ORDERS[0][:4] + _ORDERS[1][:4]

        def expM(gk_sub, gq_sub):
            diff = gk_sub[:, None] - gq_sub[None, :]
            m = np.where(diff <= 0, np.exp(_sigmoid(diff + pbm)), 0.0)
            return m.astype(np.float32)

        parts = []
        for qi in range(4):
            gq = np.arange(order[qi] * NBQ, (order[qi] + 1) * NBQ).astype(np.float64)
            for s in MASKED[qi]:
                gblk = kv_order[s]
                gk = np.arange(gblk * NBQ, (gblk + 1) * NBQ).astype(np.float64)
                # [512, 512] -> [128, 4, 512]
                parts.append(_chunk_part(expM(gk, gq), 4)[:, None, :, :])
        mskc = np.concatenate(parts, axis=1).astype(bf16)  # [128, NMSK, 4, 512]
        in_maps.append({
            "xt": xt, "xo": xo, "wkv": wkv, "wql": wql, "wproj": wproj,
            "msk": np.ascontiguousarray(mskc),
        })
    return in_maps


def kernel(x, expand, project, position_bias_mult):
    global LAST_RESULTS
    x = np.asarray(x, dtype=np.float32)
    expand = np.asarray(expand, dtype=np.float32)
    project = np.asarray(project, dtype=np.float32)
    pbm = float(np.asarray(position_bias_mult))

    in_maps = _prep_inputs(x, expand, project, pbm)
    nc = _build_nc()
    res = run_bass_kernel_spmd(nc, in_maps, core_ids=list(range(8)))
    LAST_RESULTS = res

    full = np.empty((B, N, D), dtype=np.float32)
    for c in range(8):
        b, half = c // 2, c % 2
        order = _ORDERS[half]
        o = res.results[c]["out"]
        for qi in range(4):
            blk = order[qi]
            full[b, blk * 512:(blk + 1) * 512] = o[qi * 512:(qi + 1) * 512]
    return full


# revision 23
# speedup vs baseline: 1.0893x; 1.0893x over previous
"""Trainium2 Bass kernel for nn_AttentionLayer (B=4, N=4096, D=1024).

Reference computation:
  nx = layernorm(x)
  h  = nx @ expand                       # [B,N,4352]
  q  = h[:, :128] ; k = h[:, 128:256]
  linear = h[:, 256:2304]; pre_gelu = h[:, 2304:4352]
  gated  = linear * gelu(pre_gelu)       # exact erf gelu
  local  = gated[:, :1024]; v = gated[:, 1024:2048]
  mask[i,j] = j<=i ? sigmoid((j-i)+pbm) : -inf
  attn = softmax(q k^T / sqrt(128) + mask) @ v
  out  = x + concat([local, attn]) @ project

Sharding (8 cores, SPMD): batch b -> core pair (2b, 2b+1).  Per pair,
512-row query blocks interleave for causal load balance: even core owns
blocks {0,3,4,7}, odd owns {1,2,5,6}.  Each core computes LN + expand for
its OWN 2048 rows only; k/v of the other half arrive via pairwise
AllGathers (one k + one v collective per 512-row chunk, issued as each
chunk lands in HBM so the wire time pipelines under the remaining
expand).  The kv slot order is the fixed pair order
[even-core blocks | odd-core blocks], the same on both cores, so the
SPMD attention schedule is uniform: q-slot i attends a fixed slot set
(2/4/6/8 slots).  Causality + position bias use a host-precomputed
multiplicative mask expM = causal ? exp(sigmoid(j-i+pbm)) : 0, but the
mask is only loaded/applied on the 11 (q-slot, kv-slot) pairs where it
differs from 1.0 on either core of the pair (diagonal blocks, band
precursors, and fully-future blocks); elsewhere expM == 1 to bf16
precision because sigmoid(j-i+pbm) underflows ~16 columns past the
diagonal.  P = exp(qk)*expM is normalized by its row sum (no max
subtraction: logits are O(1) after layernorm + xavier weights).

Precision: fp8(e4m3) DoubleRow matmuls (2x PE throughput) for the
expand, attention*V and project matmuls, with per-tensor power-of-2
scales folded into psum-evacuation constants; qk^T stays bf16.  psum
accumulation is f32 throughout.  Measured end-to-end rel err ~1.2e-2
(tolerance 2e-2).

Schedule notes: all x^T tiles are preloaded up front (no per-chunk DMA
dependency chains); DMAs are issued as single multi-dim descriptors
(host pre-transposes arrays into [128, chunk, cols] layouts) to keep
the sync-engine descriptor-issue time off the critical path; psum
evacuations that feed a multiply are fused into one
scalar_tensor_tensor; the attention denominator matmul is emitted after
the first AV psum group so the PE never waits on the DVE add chain.
"""

import math

import numpy as np
import ml_dtypes

import concourse.bass as bass
import concourse.mybir as mybir
from concourse import bacc
import concourse.tile as tile
from concourse.bass_utils import run_bass_kernel_spmd

BF16 = mybir.dt.bfloat16
F32 = mybir.dt.float32
FP8 = mybir.dt.float8e4
AF = mybir.ActivationFunctionType
PM = mybir.MatmulPerfMode
MUL = mybir.AluOpType.mult
ADD = mybir.AluOpType.add

B, N, D = 4, 4096, 1024
QK = 128
E = 2048
R = N              # kv rows per core
RO = 2048          # own query rows per core
DCH = D // 128     # 8 contraction chunks
NT = 512           # matmul free-dim tile
W2 = 2176          # 128 (q or k) + 1024 (linear) + 1024 (gelu) cols

# fp8 scales (powers of 2; relative precision is scale-free, these just
# center the dynamic range away from subnormals/overflow)
SX = 16.0          # nx (post-LN activations)
SW = 256.0         # wk / wlin / wgel columns
SWQ = 4096.0       # wq columns (also absorbs the 1/sqrt(qk) prescale)
SV = 8.0           # v / local / attn (the project stationary operands)
SWP = 256.0        # wproj
LN4 = math.log(4.0)  # exp bias => P scaled by 4
USK = 1.0 / (SX * SW)    # 2^-12: k / linear / gelu psum evacuation
USQ = 1.0 / (SX * SWQ)   # 2^-16: q psum evacuation
USO = 1.0 / (SV * SWP)   # 2^-11: project psum evacuation
# NOTE: attnT = av_psum/den_psum exactly: av = sum (4P)(8v) = 32*sum(Pv),
# den = 4*sum(P), so av/den = 8*attn = SV*attn as required.

# attention schedule: q-slot qi attends kv slots 0..qi and 4..4+qi
SCHED = {0: [0, 4], 1: [0, 1, 4, 5], 2: [0, 1, 2, 4, 5, 6], 3: [0, 1, 2, 3, 4, 5, 6, 7]}
# (qi, slot) pairs whose mask differs from all-ones on either core of the
# pair: diagonal blocks, band precursors (j >= i-16 tail), future blocks
MASKED = {0: (0, 4), 1: (1, 4, 5), 2: (1, 2, 6), 3: (3, 6, 7)}
MIDX = {}
for _qi in range(4):
    for _s in MASKED[_qi]:
        MIDX[(_qi, _s)] = len(MIDX)
NMSK = len(MIDX)   # 11

LAST_RESULTS = None  # set by kernel(); test harness reads exec_time_ns


def _build_nc():
    nc = bacc.Bacc(None)

    # host pre-transposed layouts: leading dim is the SBUF partition
    xt = nc.declare_dram_parameter("xt", [128, DCH, RO], BF16, isOutput=False)
    xo = nc.declare_dram_parameter("xo", [RO, D], BF16, isOutput=False)
    wkv = nc.declare_dram_parameter("wkv", [128, DCH, W2], FP8, isOutput=False)
    wql = nc.declare_dram_parameter("wql", [128, DCH, W2], FP8, isOutput=False)
    wproj = nc.declare_dram_parameter("wproj", [128, 16, D], FP8, isOutput=False)
    msk = nc.declare_dram_parameter("msk", [128, NMSK, 4, NT], BF16, isOutput=False)
    out = nc.declare_dram_parameter("out", [RO, D], F32, isOutput=True)

    with tile.TileContext(nc) as tc:
        with tc.tile_pool(name="const", bufs=1) as cpool:
            ones128 = cpool.tile([128, 1], BF16)
            nc.vector.memset(ones128[:], 1.0)
            ones1 = cpool.tile([1, 128], BF16)
            nc.vector.memset(ones1[:], 1.0)
            ln4b = cpool.tile([128, 1], F32)
            nc.vector.memset(ln4b[:], LN4)
            epsb = cpool.tile([128, 1], F32)
            nc.vector.memset(epsb[:], 1e-5 / (SX * SX))

            with tc.tile_pool(name="dram", bufs=1, space="DRAM") as dpool:
                kv_own = [dpool.tile([128, 4, 1024], FP8, name=f"kv_own_{r}")
                          for r in range(4)]
                kv_all = [dpool.tile([2 * 128, 4, 1024], FP8, name=f"kv_all_{r}")
                          for r in range(4)]
                k_own = [dpool.tile([128, NT], BF16, name=f"k_own_{r}")
                         for r in range(4)]
                k_all = [dpool.tile([2 * 128, NT], BF16, name=f"k_all_{r}")
                         for r in range(4)]

                with tc.tile_pool(name="persist", bufs=1) as ppool, \
                     tc.tile_pool(name="attnT_p", bufs=1) as apool, \
                     tc.tile_pool(name="q0_p", bufs=1) as q0pool:
                    kT_sb = ppool.tile([128, R], BF16)         # k^T, hT layout
                    qT_sb = ppool.tile([128, RO], BF16)        # q^T
                    localT_sb = ppool.tile([128, 8, RO], FP8)  # [lc][128, 2048] *SV

                    # attention-phase SBUF pools (psbp/astream/awork) are the
                    # outer-scope pools above so q-slot 0's psb production can
                    # be woven into loop2 (its qk matmuls borrow the expand
                    # psum pool)
                    def make_producer(qi, pools):
                        """psb production for q-slot qi: per tile one qk
                        matmul + exp (+mask mul) + split den add.  Returned
                        as (state, generator) so the caller can weave single
                        tiles of production between other matmul groups: the
                        PE then always has work while the scalar-engine exp
                        chain paces production."""
                        kr_slots = SCHED[qi]
                        qcol = qi * NT
                        psb_pool, pt_pool, pt_tag, pt_bufs, pe_pool, mt_pool = pools
                        st = {
                            "psb": psb_pool.tile([128, len(kr_slots) * 4, NT], FP8,
                                                 tag=f"psb{qi}", bufs=1,
                                                 name=f"psb_{qi}"),
                            "den_a": psb_pool.tile([128, NT], BF16, tag=f"dena{qi}",
                                                   bufs=1, name=f"den_a_{qi}"),
                            "den_b": psb_pool.tile([128, NT], BF16, tag=f"denb{qi}",
                                                   bufs=1, name=f"den_b_{qi}"),
                        }

                        def gen():
                            psb = st["psb"]
                            for i, krs in enumerate(kr_slots):
                                mt4 = None
                                if (qi, krs) in MIDX:
                                    mt4 = mt_pool.tile([128, 4, NT], BF16,
                                                       tag="mt", bufs=4)
                                    nc.sync.dma_start(
                                        mt4[:], msk[:, MIDX[(qi, krs)], :, :])
                                for j in range(4):
                                    ti = i * 4 + j
                                    kr0 = krs * NT + j * 128
                                    pt_ps = pt_pool.tile([128, NT], F32,
                                                         tag=pt_tag, bufs=pt_bufs)
                                    nc.tensor.matmul(pt_ps[:],
                                                     kT_sb[:, kr0:kr0 + 128],
                                                     qT_sb[:, qcol:qcol + NT],
                                                     start=True, stop=True)
                                    if mt4 is not None:
                                        pe = pe_pool.tile([128, NT], BF16,
                                                          tag="pe", bufs=4)
                                        nc.scalar.activation(pe[:], pt_ps[:],
                                                             AF.Exp, bias=ln4b[:])
                                        nc.vector.tensor_mul(psb[:, ti, :], pe[:],
                                                             mt4[:, j, :])
                                    else:
                                        nc.scalar.activation(psb[:, ti, :],
                                                             pt_ps[:], AF.Exp,
                                                             bias=ln4b[:])
                                    acc = st["den_a"] if ti % 2 == 0 else st["den_b"]
                                    if ti < 2:
                                        nc.vector.tensor_copy(acc[:], psb[:, ti, :])
                                    else:
                                        nc.vector.tensor_add(acc[:], acc[:],
                                                             psb[:, ti, :])
                                    yield
                        return st, gen()

                    def weave(nxt, k):
                        if nxt is None:
                            return
                        for _ in range(k):
                            if next(nxt, "DONE") == "DONE":
                                break

                    # ---------------- Phase 1+2: expand ----------------
                    with tc.tile_pool(name="xt_p", bufs=1) as xtp, \
                         tc.tile_pool(name="wkv_p", bufs=1) as wkvp, \
                         tc.tile_pool(name="wql_p", bufs=1) as wqlp, \
                         tc.tile_pool(name="ex_stream", bufs=4) as estream, \
                         tc.tile_pool(name="ex_work", bufs=3) as ework, \
                         tc.tile_pool(name="st_work", bufs=2) as swork, \
                         tc.tile_pool(name="ex_psum", bufs=5, space="PSUM") as epsum, \
                         tc.tile_pool(name="st_psum", bufs=2, space="PSUM") as spsum:
                        # preload ALL x^T tiles + weights up front; chunk 0's x
                        # first (feeds the first stats chain), then wkv (first
                        # matmuls), remaining x, then wql (needed only in loop2)
                        xt_all = [xtp.tile([128, DCH, NT], BF16, name=f"xt_{r}")
                                  for r in range(4)]
                        wkv_sb = wkvp.tile([128, DCH, W2], FP8)
                        wql_sb = wqlp.tile([128, DCH, W2], FP8)
                        for h in range(2):
                            nc.sync.dma_start(xt_all[0][:, 4 * h:4 * h + 4, :],
                                              xt[:, 4 * h:4 * h + 4, 0:NT])
                        for dq in range(4):
                            nc.sync.dma_start(wkv_sb[:, 2 * dq:2 * dq + 2, :],
                                              wkv[:, 2 * dq:2 * dq + 2, :])
                        for r in range(1, 4):
                            for h in range(2):
                                nc.sync.dma_start(
                                    xt_all[r][:, 4 * h:4 * h + 4, :],
                                    xt[:, 4 * h:4 * h + 4, r * NT:(r + 1) * NT])
                        for dq in range(4):
                            nc.sync.dma_start(wql_sb[:, 2 * dq:2 * dq + 2, :],
                                              wql[:, 2 * dq:2 * dq + 2, :])

                        def stats_chain(rch):
                            """Raw LN sums for chunk rch from the preloaded x^T
                            tiles: partition-sum matmuls + scalar-engine psum
                            evacuation.  Emitted one iteration ahead so the DVE
                            adds hide under the previous chunk's expand."""
                            xts = xt_all[rch]
                            mu_ps = spsum.tile([1, NT], F32, tag="stat", name=f"mu_ps_{rch}")
                            sq_ps = spsum.tile([1, NT], F32, tag="stat", name=f"sq_ps_{rch}")
                            # accumulate the 8 d-chunks on DVE (bf16 2x mode), then a
                            # single partition-sum matmul per stat instead of 8 each
                            acc_mu = estream.tile([128, NT], BF16, tag="acc_mu", bufs=2,
                                                  name=f"accmu_{rch}")
                            acc_sq = estream.tile([128, NT], BF16, tag="acc_sq", bufs=2,
                                                  name=f"accsq_{rch}")
                            sq_prev = estream.tile([128, NT], BF16, tag="sq_s", bufs=2,
                                                    name=f"sq_{rch}_0")
                            nc.vector.tensor_mul(sq_prev[:], xts[:, 0, :], xts[:, 0, :])
                            nc.vector.tensor_add(acc_mu[:], xts[:, 0, :], xts[:, 1, :])
                            for dch in range(1, DCH):
                                sqt = estream.tile([128, NT], BF16, tag="sq_s", bufs=2,
                                                   name=f"sq_{rch}_{dch}")
                                nc.vector.tensor_mul(sqt[:], xts[:, dch, :], xts[:, dch, :])
                                if dch == 1:
                                    nc.vector.tensor_add(acc_sq[:], sq_prev[:], sqt[:])
                                else:
                                    nc.vector.tensor_add(acc_sq[:], acc_sq[:], sqt[:])
                                if dch >= 2:
                                    nc.vector.tensor_add(acc_mu[:], acc_mu[:], xts[:, dch, :])
                            nc.tensor.matmul(mu_ps[:], ones128[:], acc_mu[:],
                                             start=True, stop=True)
                            nc.tensor.matmul(sq_ps[:], ones128[:], acc_sq[:],
                                             start=True, stop=True)
                            mu16 = swork.tile([1, NT], BF16, tag="st_m16", bufs=1, name=f"m16_{rch}")
                            e16 = swork.tile([1, NT], BF16, tag="st_e16", bufs=1, name=f"e16_{rch}")
                            nc.vector.tensor_scalar_mul(mu16[:], mu_ps[:], 1.0 / D)
                            nc.vector.tensor_scalar_mul(e16[:], sq_ps[:], 1.0 / D)
                            return mu16, e16

                        def bcast_chain(rch, mu16, e16):
                            # broadcast the raw mean / second moment to 128
                            # partitions FIRST, then do the LN scale/shift math
                            # at [128,NT] width (DVE [1,N] ops cost the same
                            # cycles as [128,N] ones, so broadcasting early is
                            # free and keeps the chain short)
                            bps = spsum.tile([128, NT], F32, tag="bcast", bufs=1, name=f"bps_{rch}")
                            nc.tensor.matmul(bps[:], ones1[:], mu16[:], start=True, stop=True)
                            mub = swork.tile([128, NT], BF16, tag="mub", bufs=2,
                                             name=f"mub_{rch}")
                            nc.vector.tensor_copy(mub[:], bps[:])
                            bps2 = spsum.tile([128, NT], F32, tag="bcast", bufs=1, name=f"bps2_{rch}")
                            nc.tensor.matmul(bps2[:], ones1[:], e16[:], start=True, stop=True)
                            e2b = swork.tile([128, NT], BF16, tag="e2b", bufs=2,
                                             name=f"e2b_{rch}")
                            nc.vector.tensor_copy(e2b[:], bps2[:])
                            var = swork.tile([128, NT], BF16, tag="var", bufs=1,
                                             name=f"var_{rch}")
                            nc.vector.tensor_mul(var[:], mub[:], mub[:])
                            nc.vector.tensor_sub(var[:], e2b[:], var[:])
                            # rstd_bt = SX/sqrt(var+eps) = 1/sqrt(var/SX^2 + eps/SX^2)
                            s_f = swork.tile([128, NT], F32, tag="s_f", bufs=1,
                                             name=f"s_f_{rch}")
                            nc.scalar.activation(s_f[:], var[:], AF.Sqrt,
                                                 scale=1.0 / (SX * SX), bias=epsb[:])
                            r_f = swork.tile([128, NT], F32, tag="r_f", bufs=1,
                                             name=f"r_f_{rch}")
                            nc.vector.reciprocal_approx_fast(r_f[:], s_f[:])
                            rstd_bt = swork.tile([128, NT], BF16, tag="rbt", bufs=2,
                                                 name=f"rbt_{rch}")
                            nc.vector.tensor_copy(rstd_bt[:], r_f[:])
                            sneg_bt = swork.tile([128, NT], BF16, tag="sbt", bufs=2,
                                                 name=f"sbt_{rch}")
                            nc.vector.scalar_tensor_tensor(
                                sneg_bt[:], mub[:], -1.0, rstd_bt[:], op0=MUL, op1=MUL)
                            return rstd_bt, sneg_bt

                        def center_chain(rch, rstd_bt, sneg_bt):
                            # xpp = SX*(x*rstd - mu*rstd) in fp8, [128, DCH, NT];
                            # emitted mid-way through the PREVIOUS chunk's expand
                            xts = xt_all[rch]
                            xpp = estream.tile([128, DCH, NT], FP8, tag="xpp", bufs=4,
                                               name=f"xpp_{rch}")
                            for dch in range(DCH):
                                xc = ework.tile([128, NT], BF16, tag="cen", bufs=3,
                                                name=f"cen_{rch}_{dch}")
                                nc.vector.tensor_mul(xc[:], xts[:, dch, :], rstd_bt[:])
                                nc.vector.tensor_add(xpp[:, dch, :], xc[:], sneg_bt[:])
                            return xpp

                        def v_group(rch, xpp, ms):
                            for m in ms:
                                vgel = ework.tile([128, E // 2], BF16, tag="vgel")
                                vv = ework.tile([128, E // 2], FP8, tag="vv")
                                # gelu columns first so the fused lin*gelu stt
                                # has its second operand ready
                                for vc in (2, 3, 0, 1):
                                    vps = epsum.tile([128, NT], F32, tag="mm")
                                    if vc < 2:
                                        woff = 128 + vc * NT
                                    else:
                                        woff = 1152 + (vc - 2) * NT
                                    for dp in range(DCH // 2):
                                        nc.tensor.matmul(
                                            vps[:],
                                            xpp[:, 2 * dp:2 * dp + 2, m * 128:(m + 1) * 128],
                                            wkv_sb[:, 2 * dp:2 * dp + 2, woff:woff + NT],
                                            start=(dp == 0), stop=(dp == DCH // 2 - 1),
                                            perf_mode=PM.DoubleRow)
                                    if vc >= 2:
                                        nc.scalar.activation(vgel[:, (vc - 2) * NT:(vc - 1) * NT],
                                                             vps[:], AF.Gelu, scale=USK)
                                    else:
                                        # vv = (lin_psum * USK*SV) * gelu, fused
                                        nc.vector.scalar_tensor_tensor(
                                            vv[:, vc * NT:(vc + 1) * NT], vps[:],
                                            USK * SV, vgel[:, vc * NT:(vc + 1) * NT],
                                            op0=MUL, op1=MUL)
                                nc.sync.dma_start(kv_own[rch][:, m, :], vv[:])

                        rch_order = [0, 1, 2, 3]

                        st0 = stats_chain(rch_order[0])
                        bt0 = bcast_chain(rch_order[0], st0[0], st0[1])
                        xpp_stash = center_chain(rch_order[0], bt0[0], bt0[1])
                        xpps = {}
                        NOWN = 4
                        for oi, rch in enumerate(rch_order):
                            xpp = xpp_stash
                            xpps[rch] = xpp
                            if oi + 1 < NOWN:
                                nxt = stats_chain(rch_order[oi + 1])
                            # k^T own (hT layout) -> DRAM bounce for the AllGather
                            kps = epsum.tile([128, NT], F32, tag="mm")
                            for dp in range(DCH // 2):
                                nc.tensor.matmul(kps[:],
                                                 wkv_sb[:, 2 * dp:2 * dp + 2, 0:128],
                                                 xpp[:, 2 * dp:2 * dp + 2, :],
                                                 start=(dp == 0), stop=(dp == DCH // 2 - 1),
                                                 perf_mode=PM.DoubleRow)
                            kout = ework.tile([128, NT], BF16, tag="kout", bufs=2,
                                              name=f"kout_{rch}")
                            nc.vector.tensor_scalar_mul(kout[:], kps[:], USK)
                            nc.sync.dma_start(k_own[rch][:], kout[:])
                            v_group(rch, xpp, (0, 1))
                            if oi + 1 < NOWN:
                                nbt = bcast_chain(rch_order[oi + 1], nxt[0], nxt[1])
                                xpp_stash = center_chain(rch_order[oi + 1],
                                                         nbt[0], nbt[1])
                            v_group(rch, xpp, (2, 3))
                            # AllGathers for this chunk's k and v: issued as soon as
                            # the chunk is in HBM so the wire time pipelines under
                            # the remaining expand
                            nc.gpsimd.collective_compute(
                                "AllGather",
                                mybir.AluOpType.bypass,
                                replica_groups=[[0, 1], [2, 3], [4, 5], [6, 7]],
                                ins=[k_own[rch].opt()],
                                outs=[k_all[rch].opt()],
                            )
                            nc.gpsimd.collective_compute(
                                "AllGather",
                                mybir.AluOpType.bypass,
                                replica_groups=[[0, 1], [2, 3], [4, 5], [6, 7]],
                                ins=[kv_own[rch].opt()],
                                outs=[kv_all[rch].opt()],
                            )
                            # kv slot order is [rank0 chunks | rank1 chunks]
                            nc.sync.dma_start(kT_sb[:, rch * NT:(rch + 1) * NT],
                                              k_all[rch][0:128, :])
                            nc.sync.dma_start(kT_sb[:, (4 + rch) * NT:(5 + rch) * NT],
                                              k_all[rch][128:256, :])
                        # loop2: q + local expand (covers the AllGather latency)
                        # q-slot 0's psb production weaves into loop2 chunks 1-3
                        # (qT slot 0 is written by chunk 0's q evacuation);
                        # its qk matmuls borrow the expand psum pool
                        st0, pgen0 = make_producer(
                            0, (q0pool, epsum, "mm", 5, ework, estream))
                        for rch in rch_order:
                            rs = rch * NT
                            xpp = xpps[rch]
                            qps = epsum.tile([128, NT], F32, tag="mm")
                            for dp in range(DCH // 2):
                                nc.tensor.matmul(qps[:],
                                                 wql_sb[:, 2 * dp:2 * dp + 2, 0:128],
                                                 xpp[:, 2 * dp:2 * dp + 2, :],
                                                 start=(dp == 0), stop=(dp == DCH // 2 - 1),
                                                 perf_mode=PM.DoubleRow)
                            nc.vector.tensor_scalar_mul(qT_sb[:, rs:rs + NT],
                                                        qps[:], USQ)
                            if rch == 3:
                                weave(pgen0, 99)
                            for lc in range(8):
                                lps = epsum.tile([128, NT], F32, tag="mm")
                                gps = epsum.tile([128, NT], F32, tag="mm")
                                for dp in range(DCH // 2):
                                    nc.tensor.matmul(
                                        gps[:],
                                        wql_sb[:, 2 * dp:2 * dp + 2,
                                               1152 + lc * 128:1280 + lc * 128],
                                        xpp[:, 2 * dp:2 * dp + 2, :],
                                        start=(dp == 0), stop=(dp == DCH // 2 - 1),
                                        perf_mode=PM.DoubleRow)
                                for dp in range(DCH // 2):
                                    nc.tensor.matmul(
                                        lps[:],
                                        wql_sb[:, 2 * dp:2 * dp + 2,
                                               128 + lc * 128:256 + lc * 128],
                                        xpp[:, 2 * dp:2 * dp + 2, :],
                                        start=(dp == 0), stop=(dp == DCH // 2 - 1),
                                        perf_mode=PM.DoubleRow)
                                lgel = ework.tile([128, NT], BF16, tag="lgel")
                                nc.scalar.activation(lgel[:], gps[:], AF.Gelu, scale=USK)
                                # localT = (lin_psum * USK*SV) * gelu, fused
                                nc.vector.scalar_tensor_tensor(
                                    localT_sb[:, lc, rs:rs + NT], lps[:], USK * SV,
                                    lgel[:], op0=MUL, op1=MUL)
                        # finish any q-slot 0 production not woven into loop2
                        # (still inside the expand pools: its qk matmuls use
                        # the expand psum ring)
                        weave(pgen0, 99)

                    # ---------------- Phase 3: attention ----------------
                    # allocated only now so the expand phase has the SBUF;
                    # project weights prefetch during attention
                    attnT_sb = apool.tile([128, 8, RO], FP8)   # [vc][128, 2048] *SV
                    proj_sb = apool.tile([128, 16, D], FP8)
                    for pq in range(4):
                        nc.sync.dma_start(proj_sb[:, 4 * pq:4 * pq + 4, :],
                                          wproj[:, 4 * pq:4 * pq + 4, :])
                    with tc.tile_pool(name="psb_p", bufs=1) as psbp, \
                         tc.tile_pool(name="at_stream", bufs=6) as astream, \
                         tc.tile_pool(name="at_work", bufs=2) as awork, \
                         tc.tile_pool(name="at_psum", bufs=4, space="PSUM") as apsum, \
                         tc.tile_pool(name="av_psum", bufs=2, space="PSUM") as avpsum:

                        def consume(qi, st, nxt):
                            kr_slots = SCHED[qi]
                            qcol = qi * NT
                            nkr = len(kr_slots) * 4
                            psb = st["psb"]
                            nslots = len(kr_slots)
                            # weave qi+1's production into the AV groups:
                            # lightly during g0 (so the den->reciprocal chain
                            # isn't stuck behind woven DVE work), heavier in g1
                            kw0 = 2 if qi < 3 else 0
                            kw1 = 4 if qi < 3 else 0
                            rd_b = None
                            for g in range(2):
                                avs = [avpsum.tile([128, NT], F32, tag="av", bufs=4,
                                                   name=f"av{g}_{_i}")
                                       for _i in range(4)]
                                for i, krs in enumerate(kr_slots):
                                    gslot, vrank = krs % 4, krs // 4
                                    vt4 = astream.tile([128, 4, NT], FP8, tag="vt",
                                                       bufs=6)
                                    nc.sync.dma_start(
                                        vt4[:],
                                        kv_all[gslot][vrank * 128:(vrank + 1) * 128,
                                                      :, g * NT:(g + 1) * NT])
                                    for jp in range(2):
                                        ti0 = i * 4 + 2 * jp
                                        for v4 in range(4):
                                            nc.tensor.matmul(
                                                avs[v4][:],
                                                vt4[:, 2 * jp:2 * jp + 2,
                                                    v4 * 128:(v4 + 1) * 128],
                                                psb[:, ti0:ti0 + 2, :],
                                                start=(ti0 == 0), stop=(ti0 == nkr - 2),
                                                perf_mode=PM.DoubleRow)
                                    weave(nxt, kw0 if g == 0 else kw1)
                                if g == 0:
                                    den_ps = apsum.tile([1, NT], F32, tag="den",
                                                        bufs=1)
                                    nc.tensor.matmul(den_ps[:], ones128[:],
                                                     st["den_a"][:],
                                                     start=True, stop=False)
                                    nc.tensor.matmul(den_ps[:], ones128[:],
                                                     st["den_b"][:],
                                                     start=False, stop=True)
                                    # den evacuation + rd broadcast evacuation on
                                    # the scalar engine: keeps the reciprocal's
                                    # feed/drain off the busy DVE queue
                                    den = awork.tile([1, NT], F32, tag="den_sb")
                                    rec = awork.tile([1, NT], F32, tag="rec")
                                    rec16 = awork.tile([1, NT], BF16, tag="rec16")
                                    nc.vector.tensor_copy(den[:], den_ps[:])
                                    nc.vector.reciprocal_approx_fast(rec[:], den[:])
                                    nc.vector.tensor_copy(rec16[:], rec[:])
                                    weave(nxt, 3)
                                    rb_ps = apsum.tile([128, NT], F32, tag="pt",
                                                       bufs=3)
                                    nc.tensor.matmul(rb_ps[:], ones1[:], rec16[:],
                                                     start=True, stop=True)
                                    rd_b = awork.tile([128, NT], BF16, tag="rd_b")
                                    nc.vector.tensor_copy(rd_b[:], rb_ps[:])
                                for v4 in range(4):
                                    nc.vector.tensor_mul(
                                        attnT_sb[:, g * 4 + v4, qcol:qcol + NT],
                                        avs[v4][:], rd_b[:])
                            weave(nxt, 99)  # drain any remainder

                        st = st0
                        for qi in range(4):
                            nxt_st = nxt_gen = None
                            if qi < 3:
                                nxt_st, nxt_gen = make_producer(
                                    qi + 1, (psbp, apsum, "pt", 3, awork, astream))
                            consume(qi, st, nxt_gen)
                            st = nxt_st

                    # ---------------- Phase 4: project + residual ----------------
                    with tc.tile_pool(name="pr_stream", bufs=4) as prstream, \
                         tc.tile_pool(name="pr_psum", bufs=4, space="PSUM") as prpsum:
                        for rt in range(RO // 128):
                            xo_t = prstream.tile([128, D], BF16, tag="xo")
                            nc.sync.dma_start(xo_t[:], xo[rt * 128:(rt + 1) * 128, :])
                            ot = prstream.tile([128, D], F32, tag="ot")
                            for dc in range(2):
                                ops = prpsum.tile([128, NT], F32, tag="out")
                                for lp in range(4):
                                    nc.tensor.matmul(
                                        ops[:],
                                        localT_sb[:, 2 * lp:2 * lp + 2,
                                                  rt * 128:(rt + 1) * 128],
                                        proj_sb[:, 2 * lp:2 * lp + 2,
                                                dc * NT:(dc + 1) * NT],
                                        start=(lp == 0), stop=False,
                                        perf_mode=PM.DoubleRow)
                                for ap_ in range(4):
                                    nc.tensor.matmul(
                                        ops[:],
                                        attnT_sb[:, 2 * ap_:2 * ap_ + 2,
                                                 rt * 128:(rt + 1) * 128],
                                        proj_sb[:, 8 + 2 * ap_:10 + 2 * ap_,
                                                dc * NT:(dc + 1) * NT],
                                        start=False, stop=(ap_ == 3),
                                        perf_mode=PM.DoubleRow)
                                nc.vector.scalar_tensor_tensor(
                                    ot[:, dc * NT:(dc + 1) * NT], ops[:], USO,
                                    xo_t[:, dc * NT:(dc + 1) * NT],
                                    op0=MUL, op1=ADD)
                                nc.sync.dma_start(
                                    out[rt * 128:(rt + 1) * 128, dc * NT:(dc + 1) * NT],
                                    ot[:, dc * NT:(dc + 1) * NT])

    nc.compile()
    return nc


_ORDERS = {0: [0, 3, 4, 7, 1, 2, 5, 6], 1: [1, 2, 5, 6, 0, 3, 4, 7]}


def _sigmoid(x):
    return np.where(x >= 0, 1.0 / (1.0 + np.exp(-np.abs(x))),
                    np.exp(-np.abs(x)) / (1.0 + np.exp(-np.abs(x))))


def _chunk_part(a, nch):
    """[nch*128, C] -> [128, nch, C] with [p, i, c] = a[i*128+p, c]."""
    return np.ascontiguousarray(
        a.reshape(nch, 128, a.shape[1]).transpose(1, 0, 2))


def _prep_inputs(x, expand, project, pbm):
    """Build per-core input maps (host-side sharding)."""
    bf16 = ml_dtypes.bfloat16
    fp8 = ml_dtypes.float8_e4m3
    sc = 1.0 / math.sqrt(QK)
    wq = expand[:, :QK] * (sc * SWQ)
    wk = expand[:, QK:2 * QK] * SW
    lin = expand[:, 2 * QK:2 * QK + E] * SW
    gel = expand[:, 2 * QK + E:] * SW
    wkv = _chunk_part(
        np.concatenate([wk, lin[:, D:], gel[:, D:]], axis=1), DCH).astype(fp8)
    wql = _chunk_part(
        np.concatenate([wq, lin[:, :D], gel[:, :D]], axis=1), DCH).astype(fp8)
    wproj = _chunk_part(project * SWP, 16).astype(fp8)

    in_maps = []
    NBQ = 512
    for c in range(8):
        b, half = c // 2, c % 2
        order = _ORDERS[half]
        xb = x[b]
        xperm = np.concatenate([xb[blk * NBQ:(blk + 1) * NBQ] for blk in order[:4]], axis=0)
        xt = _chunk_part(np.ascontiguousarray(xperm.T), DCH).astype(bf16)  # [128, DCH, 2048]
        xo = np.ascontiguousarray(xperm).astype(bf16)
        # kv slots in FIXED pair order: [A blocks 0,3,4,7 | B blocks 1,2,5,6]
        kv_order = _ORDERS[0][:4] + _ORDERS[1][:4]

        def expM(gk_sub, gq_sub):
            diff = gk_sub[:, None] - gq_sub[None, :]
            m = np.where(diff <= 0, np.exp(_sigmoid(diff + pbm)), 0.0)
            return m.astype(np.float32)

        parts = []
        for qi in range(4):
            gq = np.arange(order[qi] * NBQ, (order[qi] + 1) * NBQ).astype(np.float64)
            for s in MASKED[qi]:
                gblk = kv_order[s]
                gk = np.arange(gblk * NBQ, (gblk + 1) * NBQ).astype(np.float64)
                # [512, 512] -> [128, 4, 512]
                parts.append(_chunk_part(expM(gk, gq), 4)[:, None, :, :])
        mskc = np.concatenate(parts, axis=1).astype(bf16)  # [128, NMSK, 4, 512]
        in_maps.append({
            "xt": xt, "xo": xo, "wkv": wkv, "wql": wql, "wproj": wproj,
            "msk": np.ascontiguousarray(mskc),
        })
    return in_maps


def kernel(x, expand, project, position_bias_mult):
    global LAST_RESULTS
    x = np.asarray(x, dtype=np.float32)
    expand = np.asarray(expand, dtype=np.float32)
    project = np.asarray(project, dtype=np.float32)
    pbm = float(np.asarray(position_bias_mult))

    in_maps = _prep_inputs(x, expand, project, pbm)
    nc = _build_nc()
    res = run_bass_kernel_spmd(nc, in_maps, core_ids=list(range(8)))
    LAST_RESULTS = res

    full = np.empty((B, N, D), dtype=np.float32)
    for c in range(8):
        b, half = c // 2, c % 2
        order = _ORDERS[half]
        o = res.results[c]["out"]
        for qi in range(4):
            blk = order[qi]
            full[b, blk * 512:(blk + 1) * 512] = o[qi * 512:(qi + 1) * 512]
    return full
